# revision 1
# baseline (speedup 1.0000x reference)
"""LIF spike-train kernel for Trainium2 (8 NeuronCores, SPMD data-parallel).

Recurrence per neuron over T=100 steps:
    mem_t   = DECAY * mem_{t-1} * (1 - spike_{t-1}) + x_t
    spike_t = (mem_t > THRESH)

Implemented with an UNSCALED kept-membrane carry c_t = mem_t * [mem_t <= THRESH]
and a single fused custom DVE op per step (one instruction on VectorE):

    mem_t = c_{t-1} * DECAY + x_t          \  one custom-DVE op:
    c_t   = mem_t if mem_t <= THRESH else 0 /  select((Src0*C0+Src1) <= C1, ., 0)

This is bit-exact to the reference: the DVE computes fp32 per ALU stage, so
round(round(c*0.2) + x) matches the reference's ((mem*0.2)*(1-spike)) + x
exactly (multiplication by the {0,1} mask is exact, and c==mem when kept).

The spike output is recovered as spike_t = (c_t == 0): mem_t > THRESH zeroes
the carry, and a kept carry is only zero when mem_t == +-0.0 exactly
(measure-zero for continuous inputs). It is emitted as uint8 by a 1-input
tensor_scalar(is_equal, 0.0) on the Pool engine, keeping VectorE free.

Sharding: batch 128 -> 16 per core. Per core 65536 neurons laid out as
[128 partitions, 512 neurons]; input host-transposed to [128, 100, 512]
(partition, time, neuron) so every DMA run is contiguous per partition.
Output is uint8 [128, 100, 512] per core (4x less store traffic),
converted to float32 on the host.
"""

import sys

sys.path.insert(0, "/opt/trn_rl_repo")

import numpy as np

THRESH = 0.5
DECAY = 0.2
T = 100
P = 128
F = 512  # neurons per partition per core
N_CORES = 8
B_PER_CORE = 16  # 128 / 8
TC = 10  # time steps per DMA chunk
SPIKE_ENGINE = "pool"  # "pool" | "vector"


def _register_lif_op():
    """Register the fused LIF carry-update op in dve_ops.OPS (idempotent)."""
    import concourse.dve_ops as dve_ops
    from concourse.dve_spec import C0, C1, Spec, Src0, Src1, Zero, select

    for op in dve_ops.OPS:
        if op.name == "LIF_CARRY_ANT":
            return op

    def _ref(in0, in1, s0, s1, imm2):
        mem = in0.astype(np.float32) * np.float32(s0) + in1.astype(np.float32)
        return np.where(mem <= np.float32(s1), mem, np.float32(0.0)).astype(
            np.float32
        )

    mem = Src0 * C0 + Src1
    op = dve_ops.DveOp(
        "LIF_CARRY_ANT",
        Spec(body=select(mem <= C1, mem, Zero), reference=_ref),
        subdim=False,
        uops_sha={"v3": "5687192b3dfdc689", "v4": "b477c9ba1a9a5d20"},
    )
    dve_ops.OPS.append(op)
    dve_ops.CUSTOM_DVE_SPECS[op.name] = op.spec
    dve_ops._SUB_OPCODE_FOR_NAME[op.name] = (
        dve_ops._CUSTOM_DVE_ROW_BASE + len(dve_ops.OPS) - 1
    )
    assert dve_ops._SUB_OPCODE_FOR_NAME[op.name] < 0x20
    return op


def _patch_tail_drain():
    """This container's walrus rejects >1 sync-wait on one CTRL instruction;
    spread the TileContext tail-drain waits across sync-engine NOPs."""
    from concourse import mybir, tile
    from concourse.vector_clock import ScopedClock

    if getattr(tile.TileContext, "_ant_drain_patched", False):
        return

    def _drain_and_barrier(self, tick_clock, wait_clock):
        nc = self.nc
        drain_inst = nc.sync.drain()
        wait_clock.add_sem_waits(
            drain_inst.ins, ScopedClock({None: tick_clock.global_clock})
        )
        si = drain_inst.ins.sync_info
        if si is not None and si.on_wait and len(si.on_wait) > 1:
            extra = list(si.on_wait)
            si.on_wait = []
            for i, w in enumerate(extra):
                nop = nc.sync.nop(hint=f"drain_split_{i}", nofuse=True)
                nsi = nop.ins.sync_info
                if nsi is None:
                    nop.ins.sync_info = mybir.SyncInfo(on_wait=[w], on_update=[])
                else:
                    nsi.on_wait = [w]
        nc.all_engine_barrier()
        popped = nc._tile_sem_poison_stack.pop()
        assert popped is self._sem_poison
        nc.clear_and_free_semaphores(list(self.sems.allocated().values()))
        nc.all_engine_barrier()

    tile.TileContext._drain_and_barrier = _drain_and_barrier
    tile.TileContext._ant_drain_patched = True


def _split_excess_waits(nc, max_waits=1):
    """Walrus in this container rejects instructions carrying more than a
    couple of sync waits; hoist excess waits onto same-engine NOPs placed
    immediately before the instruction (same per-engine program order)."""
    from concourse import mybir

    n_split = 0
    for fn in nc.m.functions:
        for bb in fn.blocks:
            out = []
            for ins in bb.instructions:
                si = getattr(ins, "sync_info", None)
                if si is not None and si.on_wait and len(si.on_wait) > max_waits:
                    waits = list(si.on_wait)
                    keep = waits[-max_waits:]
                    extra = waits[: -max_waits]
                    si.on_wait = keep
                    for j, w in enumerate(extra):
                        nop = mybir.InstNoOp(
                            name=f"{ins.name}-wsplit{j}",
                            engine=ins.engine,
                            bass_nofuse=True,
                            sync_info=mybir.SyncInfo(on_wait=[w], on_update=[]),
                        )
                        out.append(nop)
                        n_split += 1
                out.append(ins)
            bb.instructions = out
    return n_split


_nc_cache = None


def build_bass(reps=1):
    global _nc_cache
    if _nc_cache is not None and reps == 1:
        return _nc_cache
    from concourse import bass, mybir, tile

    _patch_tail_drain()
    lif_op = _register_lif_op()

    f32 = mybir.dt.float32
    u8 = mybir.dt.uint8
    nc = bass.Bass()
    x_ext = nc.declare_dram_parameter("x", [P, T, F], f32, isOutput=False)
    out_ext = nc.declare_dram_parameter("out", [P, T, F], u8, isOutput=True)

    with tile.TileContext(nc) as tc:
        with (
            tc.tile_pool(name="xin", bufs=3) as xin_pool,
            tc.tile_pool(name="sout", bufs=3) as sout_pool,
            tc.tile_pool(name="carry", bufs=4) as carry_pool,
        ):
          for _rep in range(reps):
            c_prev = carry_pool.tile([P, F], f32)
            nc.vector.memset(c_prev[:], 0.0)  # c_{-1} = 0
            for ci in range(T // TC):
                tin = xin_pool.tile([P, TC, F], f32)
                nc.sync.dma_start(tin[:], x_ext[:, ci * TC : (ci + 1) * TC, :])
                tout = sout_pool.tile([P, TC, F], u8)
                for tl in range(TC):
                    c_new = carry_pool.tile([P, F], f32)
                    nc.vector._custom_dve(
                        lif_op,
                        out=c_new[:],
                        in0=c_prev[:],
                        in1=tin[:, tl, :],
                        s0=DECAY,
                        s1=THRESH,
                    )
                    eng = nc.gpsimd if SPIKE_ENGINE == "pool" else nc.vector
                    eng.tensor_scalar(
                        tout[:, tl, :], c_new[:], 0.0, None, mybir.AluOpType.is_equal
                    )
                    c_prev = c_new
                nc.sync.dma_start(out_ext[:, ci * TC : (ci + 1) * TC, :], tout[:])

    _split_excess_waits(nc, max_waits=1)
    # Raw Bass skips codegen_inst_isa_subclasses; without it the custom-DVE
    # InstISA carries empty .instr bytes and walrus fails ("ISA wrong length").
    mybir.codegen_inst_isa_subclasses(nc)
    if reps == 1:
        _nc_cache = nc
    return nc


def _prep_core_input(xc):
    # xc: [16, 4096, 100] fp32 -> [128, 100, 512] (partition, time, neuron)
    return np.ascontiguousarray(xc.reshape(P, F, T).transpose(0, 2, 1))


def _unprep_core_output(oc):
    # oc: [128, 100, 512] u8 -> [16, 4096, 100] fp32
    return oc.transpose(0, 2, 1).reshape(B_PER_CORE, 4096, T).astype(np.float32)


def kernel(x, _trace=False, _trace_kwargs=None):
    from concourse.bass_utils import run_bass_kernel_spmd

    nc = build_bass()
    xs = x.reshape(N_CORES, B_PER_CORE, 4096, T)
    in_maps = [{"x": _prep_core_input(xs[c])} for c in range(N_CORES)]
    kw = {}
    if _trace:
        kw["trace"] = True
        kw.update(_trace_kwargs or {})
    res = run_bass_kernel_spmd(nc, in_maps, list(range(N_CORES)), **kw)
    out = np.concatenate(
        [_unprep_core_output(res.results[c]["out"]) for c in range(N_CORES)], axis=0
    )
    if _trace:
        return out, res
    return out



# revision 2
# speedup vs baseline: 1.8492x; 1.8492x over previous
"""LIF spike-train kernel for Trainium2 (8 NeuronCores, SPMD data-parallel).

Recurrence per neuron over T=100 steps:
    mem_t   = DECAY * mem_{t-1} * (1 - spike_{t-1}) + x_t
    spike_t = (mem_t > THRESH)

One fused custom-DVE op per step computes the whole update with the carry
encoded as y_t = mem_t if mem_t <= THRESH else 1.0:

    keep = (Src0 <= C1)                 \
    m    = Src0 * (keep * C0) + Src1     |  single DVE instruction
    y    = select(m <= C1, m, One)      /

This is bit-exact vs the reference: each ALU stage rounds fp32, the {0,0.2}
mask multiply matches ((mem*0.2)*(1-spike)) exactly, and a kept membrane is
<= 0.5 < 1.0 so spike_t == (y_t > 0.5) with no ambiguity.

Spikes are extracted once per time-chunk on the otherwise-idle Scalar
(Activation) engine as Relu(2y - 1) -> uint8 (exactly {0,1}), NOT per-step on
GPSIMD: a GPSIMD tensor_scalar costs ~7.9us on this hardware (vs ~0.8us
modeled), and 100 of them fully serialized the old kernel (~790us).

Layout per core: batch 16, neurons as [128 partitions, 512 free]; input
host-transposed to [128, 100, 512] so each chunk DMA is 128 x 20KB contiguous
runs. Output uint8 [128, 100, 512] (4x less store traffic), host converts.
Chunk schedule tapers (3,7,10x8,6,4) to shorten pipeline fill and drain.
Input DMAs ride the sync queue alone (no interleaved stores to stall the
stream); output DMAs issue from the scalar queue right after each extraction.
"""

import sys

sys.path.insert(0, "/opt/trn_rl_repo")

import numpy as np

THRESH = 0.5
DECAY = 0.2
T = 100
P = 128
F = 512
N_CORES = 8
B_PER_CORE = 16
TC = 10
CHUNKS = [3, 7] + [TC] * 8 + [6, 4]  # sums to T
XIN_BUFS = 3
Y_BUFS = 3
OUT_BUFS = 12


def _register_lif_y_op():
    """Register the fused LIF y-update op in dve_ops.OPS (idempotent)."""
    import concourse.dve_ops as dve_ops
    from concourse.dve_spec import C0, C1, One, Spec, Src0, Src1, select

    name = "LIF_Y_ANT"
    for op in dve_ops.OPS:
        if op.name == name:
            return op

    def _ref(in0, in1, s0, s1, imm2):
        in0 = in0.astype(np.float32)
        in1 = in1.astype(np.float32)
        keep = (in0 <= np.float32(s1)).astype(np.float32)
        km = (keep * np.float32(s0)).astype(np.float32)
        m = ((in0 * km).astype(np.float32) + in1).astype(np.float32)
        return np.where(m <= np.float32(s1), m, np.float32(1.0)).astype(np.float32)

    keep = Src0 <= C1
    m = (Src0 * (keep * C0)) + Src1
    body = select(m <= C1, m, One)
    spec = Spec(body=body, reference=_ref)

    from concourse.dve_uop import DveOpSpec

    opcode = dve_ops._CUSTOM_DVE_ROW_BASE + len(dve_ops.OPS)
    assert opcode < 0x20
    shas = {}
    for ver in ("v3", "v4"):
        tmp = DveOpSpec(
            name=name,
            opcode=opcode,
            uops=dve_ops.lower(spec, ver=ver),
            rd1_en=dve_ops.has_src1(spec),
        )
        shas[ver] = tmp.sha(ver)
    op = dve_ops.DveOp(name, spec, subdim=False, uops_sha=shas)
    dve_ops.OPS.append(op)
    dve_ops.CUSTOM_DVE_SPECS[name] = spec
    dve_ops._SUB_OPCODE_FOR_NAME[name] = opcode
    return op


def _patch_tail_drain():
    """This container's walrus rejects >1 sync-wait on one CTRL instruction;
    spread the TileContext tail-drain waits across sync-engine NOPs."""
    from concourse import mybir, tile
    from concourse.vector_clock import ScopedClock

    if getattr(tile.TileContext, "_ant_drain_patched", False):
        return

    def _drain_and_barrier(self, tick_clock, wait_clock):
        nc = self.nc
        drain_inst = nc.sync.drain()
        wait_clock.add_sem_waits(
            drain_inst.ins, ScopedClock({None: tick_clock.global_clock})
        )
        si = drain_inst.ins.sync_info
        if si is not None and si.on_wait and len(si.on_wait) > 1:
            extra = list(si.on_wait)
            si.on_wait = []
            for i, w in enumerate(extra):
                nop = nc.sync.nop(hint=f"drain_split_{i}", nofuse=True)
                nsi = nop.ins.sync_info
                if nsi is None:
                    nop.ins.sync_info = mybir.SyncInfo(on_wait=[w], on_update=[])
                else:
                    nsi.on_wait = [w]
        nc.all_engine_barrier()
        popped = nc._tile_sem_poison_stack.pop()
        assert popped is self._sem_poison
        nc.clear_and_free_semaphores(list(self.sems.allocated().values()))
        nc.all_engine_barrier()

    tile.TileContext._drain_and_barrier = _drain_and_barrier
    tile.TileContext._ant_drain_patched = True


def _split_excess_waits(nc, max_waits=1):
    """Walrus in this container rejects instructions carrying more than a
    couple of sync waits; hoist excess waits onto same-engine NOPs placed
    immediately before the instruction (same per-engine program order)."""
    from concourse import mybir

    n_split = 0
    for fn in nc.m.functions:
        for bb in fn.blocks:
            out = []
            for ins in bb.instructions:
                si = getattr(ins, "sync_info", None)
                if si is not None and si.on_wait and len(si.on_wait) > max_waits:
                    waits = list(si.on_wait)
                    keep = waits[-max_waits:]
                    extra = waits[: -max_waits]
                    si.on_wait = keep
                    for j, w in enumerate(extra):
                        nop = mybir.InstNoOp(
                            name=f"{ins.name}-wsplit{j}",
                            engine=ins.engine,
                            bass_nofuse=True,
                            sync_info=mybir.SyncInfo(on_wait=[w], on_update=[]),
                        )
                        out.append(nop)
                        n_split += 1
                out.append(ins)
            bb.instructions = out
    return n_split


_nc_cache = {}


def build_bass(reps=1, serialize_reps=False):
    key = (reps, serialize_reps)
    if key in _nc_cache:
        return _nc_cache[key]
    from concourse import bass, mybir, tile

    _patch_tail_drain()
    lif_op = _register_lif_y_op()

    f32 = mybir.dt.float32
    u8 = mybir.dt.uint8
    nc = bass.Bass()
    x_ext = nc.declare_dram_parameter("x", [P, T, F], f32, isOutput=False)
    out_ext = nc.declare_dram_parameter("out", [P, T, F], u8, isOutput=True)

    starts = [sum(CHUNKS[:i]) for i in range(len(CHUNKS))]
    with tile.TileContext(nc) as tcx:
        with (
            tcx.tile_pool(name="xin", bufs=XIN_BUFS) as xin_pool,
            tcx.tile_pool(name="ys", bufs=Y_BUFS) as y_pool,
            tcx.tile_pool(name="sout", bufs=OUT_BUFS) as sout_pool,
            tcx.tile_pool(name="init", bufs=1) as init_pool,
        ):
            y_init = init_pool.tile([P, F], f32)
            nc.vector.memset(y_init[:], 0.0)
            bias_tile = init_pool.tile([P, 1], f32)
            nc.vector.memset(bias_tile[:], -1.0)
            y_prev = y_init[:]
            for rep in range(reps):
                if rep:
                    if serialize_reps:
                        # chain reps through DVE: y_init = y_last * 0.0
                        nc.vector.tensor_scalar(
                            y_init[:], y_prev, 0.0, None, mybir.AluOpType.mult
                        )
                    y_prev = y_init[:]
                for c0, ctc in zip(starts, CHUNKS):
                    tin = xin_pool.tile([P, ctc, F], f32)
                    nc.sync.dma_start(tin[:], x_ext[:, c0 : c0 + ctc, :])
                    ys = y_pool.tile([P, ctc, F], f32)
                    for tl in range(ctc):
                        nc.vector._custom_dve(
                            lif_op,
                            out=ys[:, tl, :],
                            in0=y_prev,
                            in1=tin[:, tl, :],
                            s0=DECAY,
                            s1=THRESH,
                        )
                        y_prev = ys[:, tl, :]
                    tout = sout_pool.tile([P, ctc, F], u8)
                    # spike = Relu(2y - 1): 0 for y<=0.5, exactly 1.0 at y==1
                    nc.scalar.activation(
                        tout[:],
                        ys[:],
                        mybir.ActivationFunctionType.Relu,
                        bias=bias_tile[:],
                        scale=2.0,
                    )
                    nc.scalar.dma_start(out_ext[:, c0 : c0 + ctc, :], tout[:])

    _split_excess_waits(nc, max_waits=1)
    # Raw Bass skips codegen_inst_isa_subclasses; without it the custom-DVE
    # InstISA carries empty .instr bytes and walrus fails ("ISA wrong length").
    mybir.codegen_inst_isa_subclasses(nc)
    _nc_cache[key] = nc
    return nc


def _prep_core_input(xc):
    # xc: [16, 4096, 100] fp32 -> [128, 100, 512] (partition, time, neuron)
    return np.ascontiguousarray(xc.reshape(P, F, T).transpose(0, 2, 1))


def _unprep_core_output(oc):
    # oc: [128, 100, 512] u8 -> [16, 4096, 100] fp32
    return oc.transpose(0, 2, 1).reshape(B_PER_CORE, 4096, T).astype(np.float32)


def kernel(x):
    from concourse.bass_utils import run_bass_kernel_spmd

    nc = build_bass()
    xs = x.reshape(N_CORES, B_PER_CORE, 4096, T)
    in_maps = [{"x": _prep_core_input(xs[c])} for c in range(N_CORES)]
    res = run_bass_kernel_spmd(nc, in_maps, list(range(N_CORES)))
    out = np.concatenate(
        [_unprep_core_output(res.results[c]["out"]) for c in range(N_CORES)], axis=0
    )
    return out


# revision 3
# speedup vs baseline: 1.9325x; 1.0450x over previous
"""LIF spike-train kernel for Trainium2 (8 NeuronCores, SPMD data-parallel).

Recurrence per neuron over T=100 steps:
    mem_t   = DECAY * mem_{t-1} * (1 - spike_{t-1}) + x_t
    spike_t = (mem_t > THRESH)

One fused custom-DVE op per step computes the whole update with the carry
encoded as y_t = mem_t if mem_t <= THRESH else 1.0:

    keep = (Src0 <= C1)                 \
    m    = Src0 * (keep * C0) + Src1     |  single DVE instruction
    y    = select(m <= C1, m, One)      /

This is bit-exact vs the reference: each ALU stage rounds fp32, the {0,0.2}
mask multiply matches ((mem*0.2)*(1-spike)) exactly, and a kept membrane is
<= 0.5 < 1.0 so spike_t == (y_t > 0.5) with no ambiguity.

Spikes are extracted once per time-chunk on the otherwise-idle Scalar
(Activation) engine as Relu(2y - 1) -> uint8 (exactly {0,1}), NOT per-step on
GPSIMD: a GPSIMD tensor_scalar costs ~7.9us on this hardware (vs ~0.8us
modeled), and 100 of them fully serialized the old kernel (~790us).

Layout per core: batch 16, neurons as [128 partitions, 512 free]; input
host-transposed to [128, 100, 512] so each chunk DMA is 128 x 20KB contiguous
runs. Output uint8 [128, 100, 512] (4x less store traffic), host converts.
Chunk schedule tapers (3,7,10x8,6,4) to shorten pipeline fill and drain.
Input DMAs ride the sync queue alone (no interleaved stores to stall the
stream); output DMAs issue from the scalar queue right after each extraction.
"""

import sys

sys.path.insert(0, "/opt/trn_rl_repo")

import numpy as np

THRESH = 0.5
DECAY = 0.2
T = 100
P = 128
F = 512
N_CORES = 8
B_PER_CORE = 16
TC = 10
CHUNKS = [3, 7] + [TC] * 8 + [6, 4]  # sums to T
XIN_BUFS = 3
Y_BUFS = 3
OUT_BUFS = 12


def _register_lif_y_op():
    """Register the fused LIF y-update op in dve_ops.OPS (idempotent)."""
    import concourse.dve_ops as dve_ops
    from concourse.dve_spec import C0, C1, One, Spec, Src0, Src1, select

    name = "LIF_Y_ANT"
    for op in dve_ops.OPS:
        if op.name == name:
            return op

    def _ref(in0, in1, s0, s1, imm2):
        in0 = in0.astype(np.float32)
        in1 = in1.astype(np.float32)
        keep = (in0 <= np.float32(s1)).astype(np.float32)
        km = (keep * np.float32(s0)).astype(np.float32)
        m = ((in0 * km).astype(np.float32) + in1).astype(np.float32)
        return np.where(m <= np.float32(s1), m, np.float32(1.0)).astype(np.float32)

    keep = Src0 <= C1
    m = (Src0 * (keep * C0)) + Src1
    body = select(m <= C1, m, One)
    spec = Spec(body=body, reference=_ref)

    from concourse.dve_uop import DveOpSpec

    opcode = dve_ops._CUSTOM_DVE_ROW_BASE + len(dve_ops.OPS)
    assert opcode < 0x20
    shas = {}
    for ver in ("v3", "v4"):
        tmp = DveOpSpec(
            name=name,
            opcode=opcode,
            uops=dve_ops.lower(spec, ver=ver),
            rd1_en=dve_ops.has_src1(spec),
        )
        shas[ver] = tmp.sha(ver)
    op = dve_ops.DveOp(name, spec, subdim=False, uops_sha=shas)
    dve_ops.OPS.append(op)
    dve_ops.CUSTOM_DVE_SPECS[name] = spec
    dve_ops._SUB_OPCODE_FOR_NAME[name] = opcode
    return op


def _patch_tail_drain():
    """This container's walrus rejects >1 sync-wait on one CTRL instruction;
    spread the TileContext tail-drain waits across sync-engine NOPs."""
    from concourse import mybir, tile
    from concourse.vector_clock import ScopedClock

    if getattr(tile.TileContext, "_ant_drain_patched", False):
        return

    def _drain_and_barrier(self, tick_clock, wait_clock):
        nc = self.nc
        drain_inst = nc.sync.drain()
        wait_clock.add_sem_waits(
            drain_inst.ins, ScopedClock({None: tick_clock.global_clock})
        )
        si = drain_inst.ins.sync_info
        if si is not None and si.on_wait and len(si.on_wait) > 1:
            extra = list(si.on_wait)
            si.on_wait = []
            for i, w in enumerate(extra):
                nop = nc.sync.nop(hint=f"drain_split_{i}", nofuse=True)
                nsi = nop.ins.sync_info
                if nsi is None:
                    nop.ins.sync_info = mybir.SyncInfo(on_wait=[w], on_update=[])
                else:
                    nsi.on_wait = [w]
        nc.all_engine_barrier()
        popped = nc._tile_sem_poison_stack.pop()
        assert popped is self._sem_poison
        nc.clear_and_free_semaphores(list(self.sems.allocated().values()))
        nc.all_engine_barrier()

    tile.TileContext._drain_and_barrier = _drain_and_barrier
    tile.TileContext._ant_drain_patched = True


def _split_excess_waits(nc, max_waits=1):
    """Walrus in this container rejects instructions carrying more than a
    couple of sync waits; hoist excess waits onto same-engine NOPs placed
    immediately before the instruction (same per-engine program order)."""
    from concourse import mybir

    n_split = 0
    for fn in nc.m.functions:
        for bb in fn.blocks:
            out = []
            for ins in bb.instructions:
                si = getattr(ins, "sync_info", None)
                if si is not None and si.on_wait and len(si.on_wait) > max_waits:
                    waits = list(si.on_wait)
                    keep = waits[-max_waits:]
                    extra = waits[: -max_waits]
                    si.on_wait = keep
                    for j, w in enumerate(extra):
                        nop = mybir.InstNoOp(
                            name=f"{ins.name}-wsplit{j}",
                            engine=ins.engine,
                            bass_nofuse=True,
                            sync_info=mybir.SyncInfo(on_wait=[w], on_update=[]),
                        )
                        out.append(nop)
                        n_split += 1
                out.append(ins)
            bb.instructions = out
    return n_split


_nc_cache = {}


def build_bass(reps=1, serialize_reps=False):
    key = (reps, serialize_reps)
    if key in _nc_cache:
        return _nc_cache[key]
    from concourse import bass, mybir, tile

    _patch_tail_drain()
    lif_op = _register_lif_y_op()

    f32 = mybir.dt.float32
    u8 = mybir.dt.uint8
    nc = bass.Bass()
    x_ext = nc.declare_dram_parameter("x", [P, T, F], f32, isOutput=False)
    out_ext = nc.declare_dram_parameter("out", [P, T, F], u8, isOutput=True)

    starts = [sum(CHUNKS[:i]) for i in range(len(CHUNKS))]
    with tile.TileContext(nc) as tcx:
        with (
            tcx.tile_pool(name="xin", bufs=XIN_BUFS) as xin_pool,
            tcx.tile_pool(name="ys", bufs=Y_BUFS) as y_pool,
            tcx.tile_pool(name="sout", bufs=OUT_BUFS) as sout_pool,
            tcx.tile_pool(name="init", bufs=1) as init_pool,
        ):
            y_init = init_pool.tile([P, F], f32)
            nc.vector.memset(y_init[:], 0.0)
            bias_tile = init_pool.tile([P, 1], f32)
            nc.vector.memset(bias_tile[:], -1.0)
            y_prev = y_init[:]
            for rep in range(reps):
                if rep:
                    if serialize_reps:
                        # chain reps through DVE: y_init = y_last * 0.0
                        nc.vector.tensor_scalar(
                            y_init[:], y_prev, 0.0, None, mybir.AluOpType.mult
                        )
                    y_prev = y_init[:]
                for c0, ctc in zip(starts, CHUNKS):
                    tin = xin_pool.tile([P, ctc, F], f32)
                    nc.sync.dma_start(tin[:], x_ext[:, c0 : c0 + ctc, :])
                    ys = y_pool.tile([P, ctc, F], f32)
                    for tl in range(ctc):
                        nc.vector._custom_dve(
                            lif_op,
                            out=ys[:, tl, :],
                            in0=y_prev,
                            in1=tin[:, tl, :],
                            s0=DECAY,
                            s1=THRESH,
                        )
                        y_prev = ys[:, tl, :]
                    tout = sout_pool.tile([P, ctc, F], u8)
                    # spike = Relu(2y - 1): 0 for y<=0.5, exactly 1.0 at y==1
                    nc.scalar.activation(
                        tout[:],
                        ys[:],
                        mybir.ActivationFunctionType.Relu,
                        bias=bias_tile[:],
                        scale=2.0,
                    )
                    nc.scalar.dma_start(out_ext[:, c0 : c0 + ctc, :], tout[:])

    _split_excess_waits(nc, max_waits=1)
    # Raw Bass skips codegen_inst_isa_subclasses; without it the custom-DVE
    # InstISA carries empty .instr bytes and walrus fails ("ISA wrong length").
    mybir.codegen_inst_isa_subclasses(nc)
    _nc_cache[key] = nc
    return nc


def _prep_core_input(xc):
    # xc: [16, 4096, 100] fp32 -> [128, 100, 512] (partition, time, neuron)
    return np.ascontiguousarray(xc.reshape(P, F, T).transpose(0, 2, 1))


def _unprep_core_output(oc):
    # oc: [128, 100, 512] u8 -> [16, 4096, 100] fp32
    return oc.transpose(0, 2, 1).reshape(B_PER_CORE, 4096, T).astype(np.float32)


def kernel(x):
    from concourse.bass_utils import run_bass_kernel_spmd

    nc = build_bass()
    x = np.asarray(x, dtype=np.float32)
    xs = x.reshape(N_CORES, B_PER_CORE, 4096, T)
    in_maps = [{"x": _prep_core_input(xs[c])} for c in range(N_CORES)]
    res = run_bass_kernel_spmd(nc, in_maps, list(range(N_CORES)))
    out = np.concatenate(
        [_unprep_core_output(res.results[c]["out"]) for c in range(N_CORES)], axis=0
    )
    return out
